# revision 13
# baseline (speedup 1.0000x reference)
"""MHSA over 32 independent 512-token segments, segment-parallel across 8
NeuronCores (4 segments / 2048 tokens per core, zero cross-core traffic).

v2 design (vs baseline): weights RESIDENT in SBUF as fp8 with residual
compensation; QKV + output projection run fp8e4 DoubleRow matmuls with a
3-term error-compensated expansion (x8.w8 + x8.wres + xres.w8) that keeps
end-to-end error at ~2e-3 while cutting PE cycles on the deep-K GEMMs;
S = QK^T stays f32r; A@V stays bf16; x^T via DMA-transpose (bf16, scaled
512x) instead of PE transposes.

Scale conventions: x512T = bf16(512 x^T); x8 = fp8(x512T/16) = fp8(32x);
xres8 = fp8(x512T/16 - x8); w8 = fp8(256 w); wres8 = fp8(256w - w8).
QKV psum = 8192*(x@w); exp scale folds 1/8192^2; vp = psum/8192 (bf16);
y512 = bf16(512*y) via (O'*512)*(1/Z); y8 = fp8(y512/16); proj psum =
8192*out, descaled 2^-13 at the output copy.
"""

import numpy as np

import concourse.bass as bass
import concourse.mybir as mybir
import concourse.tile as tile
from concourse.bass_utils import run_bass_kernel_spmd

F32 = mybir.dt.float32
F32R = mybir.dt.float32r
BF16 = mybir.dt.bfloat16
FP8 = mybir.dt.float8e4
EXP = mybir.ActivationFunctionType.Exp
DR = mybir.MatmulPerfMode.DoubleRow
MULT = mybir.AluOpType.mult
SUBTRACT = mybir.AluOpType.subtract

T, C, H, HD = 16384, 1024, 16, 64
NCORES = 8
TOK = T // NCORES          # 2048 tokens per core
SEG = 512                  # tokens per segment
NSEG = TOK // SEG          # 4 segments per core
ESCALE = 1.0 / (np.sqrt(HD) * 8192.0 * 8192.0)  # exp scale on raw S psum


def _split_multi_waits(nc):
    """Move extra sync waits onto same-engine NoOps (1-wait ISA limit)."""
    for fn in nc.m.functions:
        for bb in fn.blocks:
            out = []
            for inst in bb.instructions:
                si = inst.sync_info
                if si is not None and si.on_wait and len(si.on_wait) > 1:
                    waits = list(si.on_wait)
                    for j, w in enumerate(waits[:-1]):
                        nop = mybir.InstNoOp(name=f"{inst.name}-wsp{j}")
                        nop.engine = inst.engine
                        nop.sync_info = mybir.SyncInfo(on_wait=[w], on_update=[])
                        out.append(nop)
                    inst.sync_info = mybir.SyncInfo(
                        on_wait=[waits[-1]], on_update=list(si.on_update)
                    )
                out.append(inst)
            bb.instructions = out


def _build():
    nc = bass.Bass("TRN2", target_bir_lowering=False, debug=False)
    x = nc.dram_tensor("x_sh", [TOK, C], F32, kind="ExternalInput").ap()
    wa = nc.dram_tensor("w_attn", [C, 3 * C], F32, kind="ExternalInput").ap()
    wp = nc.dram_tensor("w_proj", [C, C], F32, kind="ExternalInput").ap()
    out = nc.dram_tensor("out", [TOK, C], F32, kind="ExternalOutput").ap()

    r3 = lambda ap, n: ap.rearrange("p (t n) -> p t n", n=n)

    with tile.TileContext(nc) as tc:
        with (
            tc.tile_pool(name="const", bufs=1) as cpool,
            tc.tile_pool(name="wres_p", bufs=1) as wpool,
            tc.tile_pool(name="xp", bufs=1) as xpool,
            tc.tile_pool(name="work", bufs=1) as work,
            tc.tile_pool(name="ps", bufs=1, space="PSUM") as pspool,
        ):
            ps = pspool.tile([128, 4096], F32, tag="ps", name="ps")

            # ---- constants
            onesf = cpool.tile([65, 64], F32, tag="onesf", name="onesf")
            onesr = cpool.tile([65, 64], F32R, tag="onesr", name="onesr")
            nc.vector.memset(onesf[64:65, :], 1.0)
            nc.vector.tensor_copy(onesr[64:65, :], onesf[64:65, :])

            # x pipeline tiles (double-buffered across segments)
            xT = [xpool.tile([128, 4096], BF16, tag=f"xT{b}", name=f"xT{b}") for b in range(2)]
            x8 = [xpool.tile([128, 4096], FP8, tag=f"x8{b}", name=f"x8{b}") for b in range(2)]
            xr8 = [xpool.tile([128, 4096], FP8, tag=f"xr8{b}", name=f"xr8{b}") for b in range(2)]

            def phase_a(s):
                """Load x segment s, convert to bf16*512, DMA-transpose, quantize."""
                b = s % 2
                for cch in range(4):
                    xf = work.tile([128, C], F32, tag="xf", bufs=2, name=f"xf{s}_{cch}")
                    nc.sync.dma_start(xf[:], x[SEG * s + 128 * cch: SEG * s + 128 * (cch + 1), :])
                    xb = work.tile([128, C], BF16, tag="xb", bufs=2, name=f"xb{s}_{cch}")
                    nc.gpsimd.tensor_scalar_mul(xb[:], xf[:], 512.0)
                    nc.sync.dma_start(
                        r3(xT[b], 512)[:, :, 128 * cch:128 * (cch + 1)], xb[:],
                        transpose=True)
                for hh in range(2):
                    sl = slice(2048 * hh, 2048 * (hh + 1))
                    nc.gpsimd.tensor_scalar_mul(x8[b][:, sl], xT[b][:, sl], 1.0 / 16.0)
                    nc.vector.scalar_tensor_tensor(
                        xr8[b][:, sl], xT[b][:, sl], 1.0 / 16.0, x8[b][:, sl],
                        MULT, SUBTRACT)

            phase_a(0)

            # ---- resident weights (fp8 main + fp8 residual), one-time
            w8 = wpool.tile([128, 8 * 3072], FP8, tag="w8", name="w8")
            wr8 = wpool.tile([128, 8 * 3072], FP8, tag="wr8", name="wr8")
            wpb = wpool.tile([128, 8 * 1024], BF16, tag="wpb", name="wpb")
            for t in range(8):
                for k in range(3):
                    wf = work.tile([128, 1024], F32, tag="ob", bufs=2, name=f"wf{t}_{k}")
                    sl = slice(3072 * t + 1024 * k, 3072 * t + 1024 * (k + 1))
                    nc.sync.dma_start(wf[:], wa[128 * t:128 * (t + 1), 1024 * k:1024 * (k + 1)])
                    nc.scalar.mul(w8[:, sl], wf[:], 256.0)
                    nc.vector.scalar_tensor_tensor(
                        wr8[:, sl], wf[:], 256.0, w8[:, sl], MULT, SUBTRACT)

            # persistent per-segment tiles
            qkt = [work.tile([128, 1024], F32R, tag=f"qk{i}", name=f"qk{i}") for i in range(8)]
            vp = [work.tile([128, 16 * 66], BF16, tag=f"vp{q}", name=f"vp{q}") for q in range(4)]
            for q in range(4):
                nc.vector.memset(
                    vp[q].rearrange("p (h w) -> p h w", w=66)[:, :, 64:65], 1.0)
            yb = work.tile([128, 4096], BF16, tag="yb", name="yb")
            ybst = work.tile([64, 4096], BF16, tag="ybst", name="ybst")

            def dr3(bank_ap, lhs_m, lhs_r, rhs_m, rhs_r, first, last):
                """One u-pair of the 3-term compensated DR accumulation."""
                nc.tensor.matmul(bank_ap, lhs_m, rhs_m, start=first, stop=False, perf_mode=DR)
                nc.tensor.matmul(bank_ap, lhs_r, rhs_m, start=False, stop=False, perf_mode=DR)
                nc.tensor.matmul(bank_ap, lhs_m, rhs_r, start=False, stop=last, perf_mode=DR)

            def phase_b(s):
                """QKV: Q^T,K^T (16 tiles) then V (8 tiles), fp8 DR 3-term."""
                b = s % 2
                w3, wr3 = r3(w8, 3072), r3(wr8, 3072)
                xs3, xr3 = r3(x8[b], 512), r3(xr8[b], 512)
                for wave in range(2):           # 0: Q chunks 0-7, 1: K chunks 0-7
                    for m in range(8):
                        cc = 8 * wave + m
                        bank = ps[0:128, 512 * m:512 * (m + 1)]
                        for u in range(4):
                            dr3(bank,
                                w3[:, 2 * u:2 * u + 2, 128 * cc:128 * (cc + 1)],
                                wr3[:, 2 * u:2 * u + 2, 128 * cc:128 * (cc + 1)],
                                xs3[:, 2 * u:2 * u + 2, :],
                                xr3[:, 2 * u:2 * u + 2, :],
                                u == 0, u == 3)
                        if m % 2 == 1:
                            j = 4 * wave + m // 2
                            nc.vector.tensor_copy(
                                qkt[j][:], ps[0:128, 512 * (m - 1):512 * (m + 1)])
                # V: out tile (tok chunk q, half v) = V[128q:128q+128, 512v:512v+512]
                for q in range(4):
                    for v in range(2):
                        bk = (2 * q + v) % 8
                        bank = ps[0:128, 512 * bk:512 * bk + 512]
                        for u in range(4):
                            dr3(bank,
                                xs3[:, 2 * u:2 * u + 2, 128 * q:128 * (q + 1)],
                                xr3[:, 2 * u:2 * u + 2, 128 * q:128 * (q + 1)],
                                w3[:, 2 * u:2 * u + 2, 2048 + 512 * v:2048 + 512 * (v + 1)],
                                wr3[:, 2 * u:2 * u + 2, 2048 + 512 * v:2048 + 512 * (v + 1)],
                                u == 0, u == 3)
                    nc.vector.tensor_scalar_mul(
                        vp[q].rearrange("p (h w) -> p h w", w=66)[:, :, 0:64],
                        ps[0:128, 1024 * q:1024 * q + 1024].rearrange(
                            "p (h w) -> p h w", w=64),
                        1.0 / 8192.0)

            def s_mm(h):
                """S^T for head h: keys on psum partitions, queries on free."""
                soff = 2048 * (h % 2)
                j, cj, r0 = h // 4, (h % 4) // 2, 64 * (h % 2)
                for kt in range(4):
                    nc.tensor.matmul(
                        ps[0:128, soff + 512 * kt:soff + 512 * (kt + 1)],
                        qkt[4 + j][r0:r0 + 64, 512 * cj + 128 * kt:512 * cj + 128 * (kt + 1)],
                        qkt[j][r0:r0 + 64, 512 * cj:512 * cj + 512],
                        start=True, stop=True)

            def phase_c(s):
                zz = work.tile([65, 1024], F32R, tag="zz", bufs=1, name=f"zz{s}")
                ats = [None] * 16

                def emit_s_exp(h):
                    s_mm(h)
                    at0 = work.tile([128, 2048], BF16, tag="at0", bufs=2, name=f"at{s}_{h}")
                    nc.scalar.activation(at0[:], ps[0:128, 2048 * (h % 2):2048 * (h % 2) + 2048],
                                         EXP, scale=ESCALE)
                    ats[h] = at0

                emit_s_exp(0)
                emit_s_exp(1)
                for h in range(16):
                    # A@V with ones column -> Z at partition 64
                    avoff = 1536 + 512 * (h % 2)
                    for kt in range(4):
                        nc.tensor.matmul(
                            ps[0:65, avoff:avoff + 512],
                            vp[kt][:, 66 * h:66 * h + 65],
                            ats[h][:, 512 * kt:512 * (kt + 1)],
                            start=(kt == 0), stop=(kt == 3))
                    if h % 2 == 1:
                        # pair tail: recip + 1/Z broadcast (psum banks 2560:3584),
                        # then one copy to sbuf and the normalize muls
                        with nc.allow_low_precision(reason="f32r out is bit-identical to f32"):
                            nc.vector.reciprocal(zz[64:65, :], ps[64:65, 1536:2560])
                        nc.tensor.matmul(ps[0:64, 2560:3072], onesr[64:65, :],
                                         zz[64:65, 0:512], start=True, stop=True)
                        nc.tensor.matmul(ps[0:64, 3072:3584], onesr[64:65, :],
                                         zz[64:65, 512:1024], start=True, stop=True)
                        rs = work.tile([64, 1024], F32, tag="rs", bufs=1, name=f"rs{s}_{h}")
                        nc.scalar.mul(rs[:], ps[0:64, 2560:3584], 512.0)
                        for g in (h - 1, h):
                            par = g % 2
                            t_ = g // 2
                            avg = 1536 + 512 * par
                            ydst = (yb[0:64, 512 * t_:512 * (t_ + 1)] if par == 0
                                    else ybst[:, 512 * t_:512 * (t_ + 1)])
                            nc.vector.tensor_mul(
                                ydst, ps[0:64, avg:avg + 512], rs[:, 512 * par:512 * (par + 1)])
                        # next pair's S/exp only after the tail consumed this
                        # pair's psum sets (setA bank 3 + setB banks 0-2)
                        for hn in (h + 1, h + 2):
                            if hn < 16:
                                emit_s_exp(hn)
                # relocate odd-head halves into partitions 64-127
                nc.scalar.dma_start(yb[64:128, :], ybst[:, :])

            def phase_d(s):
                y3, wp3 = r3(yb, 512), r3(wpb, 1024)
                for m in range(4):
                    for v in range(2):
                        bk = (2 * m + v) % 8
                        bank = ps[0:128, 512 * bk:512 * bk + 512]
                        for t in range(8):
                            nc.tensor.matmul(
                                bank,
                                y3[:, t, 128 * m:128 * (m + 1)],
                                wp3[:, t, 512 * v:512 * (v + 1)],
                                start=(t == 0), stop=(t == 7))
                    ob = work.tile([128, 1024], F32, tag="ob", bufs=2, name=f"ob{s}_{m}")
                    nc.scalar.mul(ob[:], ps[0:128, 1024 * m:1024 * m + 1024], 1.0 / 512.0)
                    nc.sync.dma_start(out[SEG * s + 128 * m:SEG * s + 128 * (m + 1), :], ob[:])

            # ---------------- main loop ----------------
            for s in range(NSEG):
                phase_b(s)
                if s + 1 < NSEG:
                    phase_a(s + 1)
                if s == 0:
                    for t in range(8):
                        wpf = work.tile([128, 1024], F32, tag="ob", bufs=2, name=f"wpf{t}")
                        sl = slice(1024 * t, 1024 * (t + 1))
                        nc.sync.dma_start(wpf[:], wp[128 * t:128 * (t + 1), :])
                        nc.scalar.copy(wpb[:, sl], wpf[:])
                phase_c(s)
                phase_d(s)

    _split_multi_waits(nc)
    return nc


_NC = None


def kernel(x, w_attn, w_proj, split_sections):
    global _NC
    if _NC is None:
        _NC = _build()
    x = np.ascontiguousarray(np.asarray(x, dtype=np.float32))
    w_attn = np.ascontiguousarray(np.asarray(w_attn, dtype=np.float32))
    w_proj = np.ascontiguousarray(np.asarray(w_proj, dtype=np.float32))
    in_maps = [
        {"x_sh": x[i * TOK:(i + 1) * TOK], "w_attn": w_attn, "w_proj": w_proj}
        for i in range(NCORES)
    ]
    res = run_bass_kernel_spmd(_NC, in_maps, core_ids=list(range(NCORES)))
    return np.concatenate([res.results[i]["out"] for i in range(NCORES)], axis=0)


if __name__ == "__main__":
    rng = np.random.default_rng(0)
    x = rng.standard_normal((T, C), dtype=np.float32)
    wa = (rng.standard_normal((C, 3 * C), dtype=np.float32) / np.sqrt(C)).astype(np.float32)
    wpj = (rng.standard_normal((C, C), dtype=np.float32) / np.sqrt(C)).astype(np.float32)
    y = kernel(x, wa, wpj, np.arange(1, 32) * 512)
    print("out", y.shape, y.dtype, np.abs(y).mean())


# revision 14
# speedup vs baseline: 1.1280x; 1.1280x over previous
"""MHSA over 32 independent 512-token segments, segment-parallel across 8
NeuronCores (4 segments / 2048 tokens per core, zero cross-core traffic).

v2 design (vs baseline): weights RESIDENT in SBUF as fp8 with residual
compensation; QKV + output projection run fp8e4 DoubleRow matmuls with a
3-term error-compensated expansion (x8.w8 + x8.wres + xres.w8) that keeps
end-to-end error at ~2e-3 while cutting PE cycles on the deep-K GEMMs;
S = QK^T stays f32r; A@V stays bf16; x^T via DMA-transpose (bf16, scaled
512x) instead of PE transposes.

Scale conventions: x512T = bf16(512 x^T); x8 = fp8(x512T/16) = fp8(32x);
xres8 = fp8(x512T/16 - x8); w8 = fp8(256 w); wres8 = fp8(256w - w8).
QKV psum = 8192*(x@w); exp scale folds 1/8192^2; vp = psum/8192 (bf16);
y512 = bf16(512*y) via (O'*512)*(1/Z); y8 = fp8(y512/16); proj psum =
8192*out, descaled 2^-13 at the output copy.
"""

import numpy as np

import concourse.bass as bass
import concourse.mybir as mybir
import concourse.tile as tile
from concourse.bass_utils import run_bass_kernel_spmd

F32 = mybir.dt.float32
F32R = mybir.dt.float32r
BF16 = mybir.dt.bfloat16
FP8 = mybir.dt.float8e4
EXP = mybir.ActivationFunctionType.Exp
DR = mybir.MatmulPerfMode.DoubleRow
MULT = mybir.AluOpType.mult
SUBTRACT = mybir.AluOpType.subtract

T, C, H, HD = 16384, 1024, 16, 64
NCORES = 8
TOK = T // NCORES          # 2048 tokens per core
SEG = 512                  # tokens per segment
NSEG = TOK // SEG          # 4 segments per core
ESCALE = 1.0 / (np.sqrt(HD) * 8192.0 * 8192.0)  # exp scale on raw S psum


def _split_multi_waits(nc):
    """Move extra sync waits onto same-engine NoOps (1-wait ISA limit)."""
    for fn in nc.m.functions:
        for bb in fn.blocks:
            out = []
            for inst in bb.instructions:
                si = inst.sync_info
                if si is not None and si.on_wait and len(si.on_wait) > 1:
                    waits = list(si.on_wait)
                    for j, w in enumerate(waits[:-1]):
                        nop = mybir.InstNoOp(name=f"{inst.name}-wsp{j}")
                        nop.engine = inst.engine
                        nop.sync_info = mybir.SyncInfo(on_wait=[w], on_update=[])
                        out.append(nop)
                    inst.sync_info = mybir.SyncInfo(
                        on_wait=[waits[-1]], on_update=list(si.on_update)
                    )
                out.append(inst)
            bb.instructions = out


def _build():
    nc = bass.Bass("TRN2", target_bir_lowering=False, debug=False)
    x = nc.dram_tensor("x_sh", [TOK, C], F32, kind="ExternalInput").ap()
    wa = nc.dram_tensor("w_attn", [C, 3 * C], F32, kind="ExternalInput").ap()
    wp = nc.dram_tensor("w_proj", [C, C], F32, kind="ExternalInput").ap()
    out = nc.dram_tensor("out", [TOK, C], F32, kind="ExternalOutput").ap()

    r3 = lambda ap, n: ap.rearrange("p (t n) -> p t n", n=n)

    with tile.TileContext(nc) as tc:
        with (
            tc.tile_pool(name="const", bufs=1) as cpool,
            tc.tile_pool(name="wres_p", bufs=1) as wpool,
            tc.tile_pool(name="xp", bufs=1) as xpool,
            tc.tile_pool(name="work", bufs=1) as work,
            tc.tile_pool(name="ps", bufs=1, space="PSUM") as pspool,
        ):
            ps = pspool.tile([128, 4096], F32, tag="ps", name="ps")

            # ---- constants
            onesf = cpool.tile([65, 64], F32, tag="onesf", name="onesf")
            onesr = cpool.tile([65, 64], F32R, tag="onesr", name="onesr")
            nc.vector.memset(onesf[64:65, :], 1.0)
            nc.vector.tensor_copy(onesr[64:65, :], onesf[64:65, :])

            # x pipeline tiles (double-buffered across segments)
            xT = [xpool.tile([128, 4096], BF16, tag=f"xT{b}", name=f"xT{b}") for b in range(2)]
            x8 = [xpool.tile([128, 4096], FP8, tag=f"x8{b}", name=f"x8{b}") for b in range(2)]
            xr8 = [xpool.tile([128, 4096], FP8, tag=f"xr8{b}", name=f"xr8{b}") for b in range(2)]

            def phase_a(s):
                """Load x segment s, convert to bf16*512, DMA-transpose, quantize."""
                b = s % 2
                for cch in range(4):
                    xf = work.tile([128, C], F32, tag="xf", bufs=2, name=f"xf{s}_{cch}")
                    nc.sync.dma_start(xf[:], x[SEG * s + 128 * cch: SEG * s + 128 * (cch + 1), :])
                    xb = work.tile([128, C], BF16, tag="xb", bufs=2, name=f"xb{s}_{cch}")
                    nc.gpsimd.tensor_scalar_mul(xb[:], xf[:], 512.0)
                    nc.sync.dma_start(
                        r3(xT[b], 512)[:, :, 128 * cch:128 * (cch + 1)], xb[:],
                        transpose=True)
                for hh in range(2):
                    sl = slice(2048 * hh, 2048 * (hh + 1))
                    nc.gpsimd.tensor_scalar_mul(x8[b][:, sl], xT[b][:, sl], 1.0 / 16.0)
                    nc.vector.scalar_tensor_tensor(
                        xr8[b][:, sl], xT[b][:, sl], 1.0 / 16.0, x8[b][:, sl],
                        MULT, SUBTRACT)

            phase_a(0)

            # ---- resident weights (fp8 main + fp8 residual), one-time
            w8 = wpool.tile([128, 8 * 3072], FP8, tag="w8", name="w8")
            wr8 = wpool.tile([128, 8 * 3072], FP8, tag="wr8", name="wr8")
            wpb = wpool.tile([128, 8 * 1024], BF16, tag="wpb", name="wpb")
            for t in range(8):
                for k in range(3):
                    wf = work.tile([128, 1024], F32, tag="ob", bufs=2, name=f"wf{t}_{k}")
                    sl = slice(3072 * t + 1024 * k, 3072 * t + 1024 * (k + 1))
                    nc.sync.dma_start(wf[:], wa[128 * t:128 * (t + 1), 1024 * k:1024 * (k + 1)])
                    nc.scalar.mul(w8[:, sl], wf[:], 256.0)
                    nc.vector.scalar_tensor_tensor(
                        wr8[:, sl], wf[:], 256.0, w8[:, sl], MULT, SUBTRACT)

            # persistent per-segment tiles
            qkt = [work.tile([128, 1024], F32R, tag=f"qk{i}", name=f"qk{i}") for i in range(8)]
            vp = [work.tile([128, 16 * 66], BF16, tag=f"vp{q}", name=f"vp{q}") for q in range(4)]
            for q in range(4):
                nc.vector.memset(
                    vp[q].rearrange("p (h w) -> p h w", w=66)[:, :, 64:65], 1.0)
            yb = work.tile([128, 4096], BF16, tag="yb", name="yb")
            ybst = work.tile([64, 4096], BF16, tag="ybst", name="ybst")

            def dr3(bank_ap, lhs_m, lhs_r, rhs_m, rhs_r, first, last):
                """One u-pair of the 3-term compensated DR accumulation."""
                nc.tensor.matmul(bank_ap, lhs_m, rhs_m, start=first, stop=False, perf_mode=DR)
                nc.tensor.matmul(bank_ap, lhs_r, rhs_m, start=False, stop=False, perf_mode=DR)
                nc.tensor.matmul(bank_ap, lhs_m, rhs_r, start=False, stop=last, perf_mode=DR)

            def phase_b(s):
                """QKV: Q^T,K^T (16 tiles) then V (8 tiles), fp8 DR 3-term."""
                b = s % 2
                w3, wr3 = r3(w8, 3072), r3(wr8, 3072)
                xs3, xr3 = r3(x8[b], 512), r3(xr8[b], 512)
                for wave in range(2):           # 0: Q chunks 0-7, 1: K chunks 0-7
                    for m in range(8):
                        cc = 8 * wave + m
                        bank = ps[0:128, 512 * m:512 * (m + 1)]
                        for u in range(4):
                            dr3(bank,
                                w3[:, 2 * u:2 * u + 2, 128 * cc:128 * (cc + 1)],
                                wr3[:, 2 * u:2 * u + 2, 128 * cc:128 * (cc + 1)],
                                xs3[:, 2 * u:2 * u + 2, :],
                                xr3[:, 2 * u:2 * u + 2, :],
                                u == 0, u == 3)
                        if m % 2 == 1:
                            j = 4 * wave + m // 2
                            nc.vector.tensor_copy(
                                qkt[j][:], ps[0:128, 512 * (m - 1):512 * (m + 1)])
                # V: out tile (tok chunk q, half v) = V[128q:128q+128, 512v:512v+512]
                for q in range(4):
                    for v in range(2):
                        bk = (2 * q + v) % 8
                        bank = ps[0:128, 512 * bk:512 * bk + 512]
                        for u in range(4):
                            dr3(bank,
                                xs3[:, 2 * u:2 * u + 2, 128 * q:128 * (q + 1)],
                                xr3[:, 2 * u:2 * u + 2, 128 * q:128 * (q + 1)],
                                w3[:, 2 * u:2 * u + 2, 2048 + 512 * v:2048 + 512 * (v + 1)],
                                wr3[:, 2 * u:2 * u + 2, 2048 + 512 * v:2048 + 512 * (v + 1)],
                                u == 0, u == 3)
                    nc.vector.tensor_scalar_mul(
                        vp[q].rearrange("p (h w) -> p h w", w=66)[:, :, 0:64],
                        ps[0:128, 1024 * q:1024 * q + 1024].rearrange(
                            "p (h w) -> p h w", w=64),
                        1.0 / 8192.0)

            def s_mm_half(h, half):
                """S^T half: keys chunks (2*half, 2*half+1) -> psum banks 0-3."""
                j, cj, r0 = h // 4, (h % 4) // 2, 64 * (h % 2)
                for i in range(2):
                    kt = 2 * half + i
                    nc.tensor.matmul(
                        ps[0:128, 1024 * half + 512 * i:1024 * half + 512 * (i + 1)],
                        qkt[4 + j][r0:r0 + 64, 512 * cj + 128 * kt:512 * cj + 128 * (kt + 1)],
                        qkt[j][r0:r0 + 64, 512 * cj:512 * cj + 512],
                        start=True, stop=True)

            def phase_c(s):
                zz = work.tile([65, 1024], F32R, tag="zz", bufs=1, name=f"zz{s}")
                ats = [None] * 16

                def emit_s_exp(h):
                    at0 = work.tile([128, 2048], BF16, tag="at0", bufs=2, name=f"at{s}_{h}")
                    for half in range(2):
                        s_mm_half(h, half)
                        nc.scalar.activation(
                            at0[:, 1024 * half:1024 * (half + 1)],
                            ps[0:128, 1024 * half:1024 * (half + 1)], EXP, scale=ESCALE)
                    ats[h] = at0

                emit_s_exp(0)
                emit_s_exp(1)
                for h in range(16):
                    # A@V with ones column -> Z at partition 64 (banks 4/5)
                    avoff = 2048 + 512 * (h % 2)
                    for kt in range(4):
                        nc.tensor.matmul(
                            ps[0:65, avoff:avoff + 512],
                            vp[kt][:, 66 * h:66 * h + 65],
                            ats[h][:, 512 * kt:512 * (kt + 1)],
                            start=(kt == 0), stop=(kt == 3))
                    if h % 2 == 1:
                        # pair tail: recip (AV banks 4,5) + 1/Z broadcast into
                        # banks 6,7, one scaled copy to sbuf, normalize muls
                        with nc.allow_low_precision(reason="f32r out is bit-identical to f32"):
                            nc.vector.reciprocal(zz[64:65, :], ps[64:65, 2048:3072])
                        nc.tensor.matmul(ps[0:64, 3072:3584], onesr[64:65, :],
                                         zz[64:65, 0:512], start=True, stop=True)
                        nc.tensor.matmul(ps[0:64, 3584:4096], onesr[64:65, :],
                                         zz[64:65, 512:1024], start=True, stop=True)
                        rs = work.tile([64, 1024], F32, tag="rs", bufs=2, name=f"rs{s}_{h}")
                        nc.scalar.mul(rs[:], ps[0:64, 3072:4096], 512.0)
                        for g in (h - 1, h):
                            par = g % 2
                            t_ = g // 2
                            avg = 2048 + 512 * par
                            ydst = (yb[0:64, 512 * t_:512 * (t_ + 1)] if par == 0
                                    else ybst[:, 512 * t_:512 * (t_ + 1)])
                            nc.vector.tensor_mul(
                                ydst, ps[0:64, avg:avg + 512], rs[:, 512 * par:512 * (par + 1)])
                    if h + 2 < 16:
                        emit_s_exp(h + 2)
                # relocate odd-head halves into partitions 64-127
                nc.scalar.dma_start(yb[64:128, :], ybst[:, :])

            def phase_d(s):
                y3, wp3 = r3(yb, 512), r3(wpb, 1024)
                for m in range(4):
                    for v in range(2):
                        bk = (2 * m + v) % 8
                        bank = ps[0:128, 512 * bk:512 * bk + 512]
                        for t in range(8):
                            nc.tensor.matmul(
                                bank,
                                y3[:, t, 128 * m:128 * (m + 1)],
                                wp3[:, t, 512 * v:512 * (v + 1)],
                                start=(t == 0), stop=(t == 7))
                    ob = work.tile([128, 1024], F32, tag="ob", bufs=2, name=f"ob{s}_{m}")
                    nc.scalar.mul(ob[:], ps[0:128, 1024 * m:1024 * m + 1024], 1.0 / 512.0)
                    nc.sync.dma_start(out[SEG * s + 128 * m:SEG * s + 128 * (m + 1), :], ob[:])

            # ---------------- main loop ----------------
            for s in range(NSEG):
                phase_b(s)
                if s + 1 < NSEG:
                    phase_a(s + 1)
                if s == 0:
                    for t in range(8):
                        wpf = work.tile([128, 1024], F32, tag="ob", bufs=2, name=f"wpf{t}")
                        sl = slice(1024 * t, 1024 * (t + 1))
                        nc.sync.dma_start(wpf[:], wp[128 * t:128 * (t + 1), :])
                        nc.scalar.copy(wpb[:, sl], wpf[:])
                phase_c(s)
                phase_d(s)

    _split_multi_waits(nc)
    return nc


_NC = None


def kernel(x, w_attn, w_proj, split_sections):
    global _NC
    if _NC is None:
        _NC = _build()
    x = np.ascontiguousarray(np.asarray(x, dtype=np.float32))
    w_attn = np.ascontiguousarray(np.asarray(w_attn, dtype=np.float32))
    w_proj = np.ascontiguousarray(np.asarray(w_proj, dtype=np.float32))
    in_maps = [
        {"x_sh": x[i * TOK:(i + 1) * TOK], "w_attn": w_attn, "w_proj": w_proj}
        for i in range(NCORES)
    ]
    res = run_bass_kernel_spmd(_NC, in_maps, core_ids=list(range(NCORES)))
    return np.concatenate([res.results[i]["out"] for i in range(NCORES)], axis=0)


if __name__ == "__main__":
    rng = np.random.default_rng(0)
    x = rng.standard_normal((T, C), dtype=np.float32)
    wa = (rng.standard_normal((C, 3 * C), dtype=np.float32) / np.sqrt(C)).astype(np.float32)
    wpj = (rng.standard_normal((C, C), dtype=np.float32) / np.sqrt(C)).astype(np.float32)
    y = kernel(x, wa, wpj, np.arange(1, 32) * 512)
    print("out", y.shape, y.dtype, np.abs(y).mean())


# revision 16
# speedup vs baseline: 1.2345x; 1.0945x over previous
"""MHSA over 32 independent 512-token segments, segment-parallel across 8
NeuronCores (4 segments / 2048 tokens per core, zero cross-core traffic).

v2 design (vs baseline): weights RESIDENT in SBUF as fp8 with residual
compensation; QKV + output projection run fp8e4 DoubleRow matmuls with a
3-term error-compensated expansion (x8.w8 + x8.wres + xres.w8) that keeps
end-to-end error at ~2e-3 while cutting PE cycles on the deep-K GEMMs;
S = QK^T stays f32r; A@V stays bf16; x^T via DMA-transpose (bf16, scaled
512x) instead of PE transposes.

Scale conventions: x512T = bf16(512 x^T); x8 = fp8(x512T/16) = fp8(32x);
xres8 = fp8(x512T/16 - x8); w8 = fp8(256 w); wres8 = fp8(256w - w8).
QKV psum = 8192*(x@w); exp scale folds 1/8192^2; vp = psum/8192 (bf16);
y512 = bf16(512*y) via (O'*512)*(1/Z); y8 = fp8(y512/16); proj psum =
8192*out, descaled 2^-13 at the output copy.
"""

import numpy as np

import concourse.bass as bass
import concourse.mybir as mybir
import concourse.tile as tile
from concourse.bass_utils import run_bass_kernel_spmd

F32 = mybir.dt.float32
F32R = mybir.dt.float32r
BF16 = mybir.dt.bfloat16
FP8 = mybir.dt.float8e4
EXP = mybir.ActivationFunctionType.Exp
DR = mybir.MatmulPerfMode.DoubleRow
MULT = mybir.AluOpType.mult
SUBTRACT = mybir.AluOpType.subtract

T, C, H, HD = 16384, 1024, 16, 64
NCORES = 8
TOK = T // NCORES          # 2048 tokens per core
SEG = 512                  # tokens per segment
NSEG = TOK // SEG          # 4 segments per core
ESCALE = 1.0 / (np.sqrt(HD) * 8192.0 * 8192.0)  # exp scale on raw S psum


def _split_multi_waits(nc):
    """Move extra sync waits onto same-engine NoOps (1-wait ISA limit)."""
    for fn in nc.m.functions:
        for bb in fn.blocks:
            out = []
            for inst in bb.instructions:
                si = inst.sync_info
                if si is not None and si.on_wait and len(si.on_wait) > 1:
                    waits = list(si.on_wait)
                    for j, w in enumerate(waits[:-1]):
                        nop = mybir.InstNoOp(name=f"{inst.name}-wsp{j}")
                        nop.engine = inst.engine
                        nop.sync_info = mybir.SyncInfo(on_wait=[w], on_update=[])
                        out.append(nop)
                    inst.sync_info = mybir.SyncInfo(
                        on_wait=[waits[-1]], on_update=list(si.on_update)
                    )
                out.append(inst)
            bb.instructions = out


def _build():
    nc = bass.Bass("TRN2", target_bir_lowering=False, debug=False)
    x = nc.dram_tensor("x_sh", [TOK, C], F32, kind="ExternalInput").ap()
    wa = nc.dram_tensor("w_attn", [C, 3 * C], F32, kind="ExternalInput").ap()
    wp = nc.dram_tensor("w_proj", [C, C], F32, kind="ExternalInput").ap()
    out = nc.dram_tensor("out", [TOK, C], F32, kind="ExternalOutput").ap()

    r3 = lambda ap, n: ap.rearrange("p (t n) -> p t n", n=n)

    with tile.TileContext(nc) as tc:
        with (
            tc.tile_pool(name="const", bufs=1) as cpool,
            tc.tile_pool(name="wres_p", bufs=1) as wpool,
            tc.tile_pool(name="xp", bufs=1) as xpool,
            tc.tile_pool(name="work", bufs=1) as work,
            tc.tile_pool(name="ps", bufs=1, space="PSUM") as pspool,
        ):
            ps = pspool.tile([128, 4096], F32, tag="ps", name="ps")

            # ---- constants
            onesf = cpool.tile([65, 64], F32, tag="onesf", name="onesf")
            onesr = cpool.tile([65, 64], F32R, tag="onesr", name="onesr")
            nc.vector.memset(onesf[64:65, :], 1.0)
            nc.vector.tensor_copy(onesr[64:65, :], onesf[64:65, :])

            # x pipeline tiles (double-buffered across segments)
            xT = [xpool.tile([128, 4096], BF16, tag=f"xT{b}", name=f"xT{b}") for b in range(2)]
            x8 = [xpool.tile([128, 4096], FP8, tag=f"x8{b}", name=f"x8{b}") for b in range(2)]
            xr8 = [xpool.tile([128, 4096], FP8, tag=f"xr8{b}", name=f"xr8{b}") for b in range(2)]

            def phase_a(s):
                """Load x segment s, convert to bf16*512, DMA-transpose, quantize."""
                b = s % 2
                for cch in range(4):
                    xf = work.tile([128, C], F32, tag="xf", bufs=2, name=f"xf{s}_{cch}")
                    nc.sync.dma_start(xf[:], x[SEG * s + 128 * cch: SEG * s + 128 * (cch + 1), :])
                    xb = work.tile([128, C], BF16, tag="xb", bufs=2, name=f"xb{s}_{cch}")
                    nc.gpsimd.tensor_scalar_mul(xb[:], xf[:], 512.0)
                    nc.sync.dma_start(
                        r3(xT[b], 512)[:, :, 128 * cch:128 * (cch + 1)], xb[:],
                        transpose=True)
                for hh in range(2):
                    sl = slice(2048 * hh, 2048 * (hh + 1))
                    nc.gpsimd.tensor_scalar_mul(x8[b][:, sl], xT[b][:, sl], 1.0 / 16.0)
                    nc.vector.scalar_tensor_tensor(
                        xr8[b][:, sl], xT[b][:, sl], 1.0 / 16.0, x8[b][:, sl],
                        MULT, SUBTRACT)

            phase_a(0)

            # ---- resident weights (fp8 main + fp8 residual), one-time
            w8 = wpool.tile([128, 8 * 3072], FP8, tag="w8", name="w8")
            wr8 = wpool.tile([128, 8 * 3072], FP8, tag="wr8", name="wr8")
            wpb = wpool.tile([128, 8 * 1024], BF16, tag="wpb", name="wpb")
            for t in range(8):
                for k in range(3):
                    wf = work.tile([128, 1024], F32, tag="ob", bufs=4, name=f"wf{t}_{k}")
                    sl = slice(3072 * t + 1024 * k, 3072 * t + 1024 * (k + 1))
                    q = nc.sync if (3 * t + k) % 2 == 0 else nc.scalar
                    q.dma_start(wf[:], wa[128 * t:128 * (t + 1), 1024 * k:1024 * (k + 1)])
                    nc.gpsimd.tensor_scalar_mul(w8[:, sl], wf[:], 256.0)
                    nc.vector.scalar_tensor_tensor(
                        wr8[:, sl], wf[:], 256.0, w8[:, sl], MULT, SUBTRACT)

            # persistent per-segment tiles
            qkt = [work.tile([128, 1024], F32R, tag=f"qk{i}", name=f"qk{i}") for i in range(8)]
            vp = [work.tile([128, 16 * 66], BF16, tag=f"vp{q}", name=f"vp{q}") for q in range(4)]
            for q in range(4):
                nc.vector.memset(
                    vp[q].rearrange("p (h w) -> p h w", w=66)[:, :, 64:65], 1.0)
            yb = work.tile([128, 4096], BF16, tag="yb", name="yb")
            ybst = work.tile([64, 4096], BF16, tag="ybst", name="ybst")

            def dr3(bank_ap, lhs_m, lhs_r, rhs_m, rhs_r, first, last):
                """One u-pair of the 3-term compensated DR accumulation."""
                nc.tensor.matmul(bank_ap, lhs_m, rhs_m, start=first, stop=False, perf_mode=DR)
                nc.tensor.matmul(bank_ap, lhs_r, rhs_m, start=False, stop=False, perf_mode=DR)
                nc.tensor.matmul(bank_ap, lhs_m, rhs_r, start=False, stop=last, perf_mode=DR)

            def phase_b(s):
                """QKV: Q^T,K^T (16 tiles) then V (8 tiles), fp8 DR 3-term."""
                b = s % 2
                w3, wr3 = r3(w8, 3072), r3(wr8, 3072)
                xs3, xr3 = r3(x8[b], 512), r3(xr8[b], 512)
                for wave in range(2):           # 0: Q chunks 0-7, 1: K chunks 0-7
                    for u in range(4):
                        for m in range(8):
                            cc = 8 * wave + m
                            bank = ps[0:128, 512 * m:512 * (m + 1)]
                            dr3(bank,
                                w3[:, 2 * u:2 * u + 2, 128 * cc:128 * (cc + 1)],
                                wr3[:, 2 * u:2 * u + 2, 128 * cc:128 * (cc + 1)],
                                xs3[:, 2 * u:2 * u + 2, :],
                                xr3[:, 2 * u:2 * u + 2, :],
                                u == 0, u == 3)
                    for j in range(4):
                        nc.vector.tensor_copy(
                            qkt[4 * wave + j][:], ps[0:128, 1024 * j:1024 * (j + 1)])
                # V: out tile (tok chunk q, half v) = V[128q:128q+128, 512v:512v+512]
                for q in range(4):
                    for v in range(2):
                        bk = (2 * q + v) % 8
                        bank = ps[0:128, 512 * bk:512 * bk + 512]
                        for u in range(4):
                            dr3(bank,
                                xs3[:, 2 * u:2 * u + 2, 128 * q:128 * (q + 1)],
                                xr3[:, 2 * u:2 * u + 2, 128 * q:128 * (q + 1)],
                                w3[:, 2 * u:2 * u + 2, 2048 + 512 * v:2048 + 512 * (v + 1)],
                                wr3[:, 2 * u:2 * u + 2, 2048 + 512 * v:2048 + 512 * (v + 1)],
                                u == 0, u == 3)
                    nc.vector.tensor_scalar_mul(
                        vp[q].rearrange("p (h w) -> p h w", w=66)[:, :, 0:64],
                        ps[0:128, 1024 * q:1024 * q + 1024].rearrange(
                            "p (h w) -> p h w", w=64),
                        1.0 / 8192.0)

            def s_mm(h):
                """S^T for head h into its parity set (banks 0-3 / 4-7)."""
                soff = 2048 * (h % 2)
                j, cj, r0 = h // 4, (h % 4) // 2, 64 * (h % 2)
                for kt in range(4):
                    nc.tensor.matmul(
                        ps[0:128, soff + 512 * kt:soff + 512 * (kt + 1)],
                        qkt[4 + j][r0:r0 + 64, 512 * cj + 128 * kt:512 * cj + 128 * (kt + 1)],
                        qkt[j][r0:r0 + 64, 512 * cj:512 * cj + 512],
                        start=True, stop=True)

            def phase_c(s):
                zz = work.tile([65, 1024], F32R, tag="zz", bufs=1, name=f"zz{s}")
                ats = [None] * 16

                def emit_s_exp(h):
                    s_mm(h)
                    at0 = work.tile([128, 2048], BF16, tag="at0", bufs=2, name=f"at{s}_{h}")
                    nc.scalar.activation(
                        at0[:], ps[0:128, 2048 * (h % 2):2048 * (h % 2) + 2048],
                        EXP, scale=ESCALE)
                    ats[h] = at0

                emit_s_exp(0)
                emit_s_exp(1)
                for h in range(16):
                    soff = 2048 * (h % 2)
                    # A@V with ones column -> Z at partition 64 (own set, bank 3)
                    avoff = soff + 1536
                    for kt in range(4):
                        nc.tensor.matmul(
                            ps[0:65, avoff:avoff + 512],
                            vp[kt][:, 66 * h:66 * h + 65],
                            ats[h][:, 512 * kt:512 * (kt + 1)],
                            start=(kt == 0), stop=(kt == 3))
                    # per-head tail: recip, 1/Z broadcast into own-set bank 1,
                    # scaled copy to sbuf (DVE), normalize mul
                    with nc.allow_low_precision(reason="f32r out is bit-identical to f32"):
                        nc.vector.reciprocal(zz[64:65, 512 * (h % 2):512 * (h % 2) + 512],
                                             ps[64:65, avoff:avoff + 512])
                    nc.tensor.matmul(ps[0:64, soff + 512:soff + 1024], onesr[64:65, :],
                                     zz[64:65, 512 * (h % 2):512 * (h % 2) + 512],
                                     start=True, stop=True)
                    rs = work.tile([64, 512], F32, tag="rs", bufs=2, name=f"rs{s}_{h}")
                    nc.vector.tensor_scalar_mul(rs[:], ps[0:64, soff + 512:soff + 1024], 512.0)
                    t_ = h // 2
                    ydst = (yb[0:64, 512 * t_:512 * (t_ + 1)] if h % 2 == 0
                            else ybst[:, 512 * t_:512 * (t_ + 1)])
                    nc.vector.tensor_mul(ydst, ps[0:64, avoff:avoff + 512], rs[:])
                    if h + 2 < 16:
                        emit_s_exp(h + 2)
                # relocate odd-head halves into partitions 64-127
                nc.scalar.dma_start(yb[64:128, :], ybst[:, :])

            def phase_d(s):
                y3, wp3 = r3(yb, 512), r3(wpb, 1024)
                for m in range(4):
                    for v in range(2):
                        bk = (2 * m + v) % 8
                        bank = ps[0:128, 512 * bk:512 * bk + 512]
                        for t in range(8):
                            nc.tensor.matmul(
                                bank,
                                y3[:, t, 128 * m:128 * (m + 1)],
                                wp3[:, t, 512 * v:512 * (v + 1)],
                                start=(t == 0), stop=(t == 7))
                    ob = work.tile([128, 1024], F32, tag="ob", bufs=4, name=f"ob{s}_{m}")
                    nc.scalar.mul(ob[:], ps[0:128, 1024 * m:1024 * m + 1024], 1.0 / 512.0)
                    nc.sync.dma_start(out[SEG * s + 128 * m:SEG * s + 128 * (m + 1), :], ob[:])

            # ---------------- main loop ----------------
            for s in range(NSEG):
                phase_b(s)
                if s + 1 < NSEG:
                    phase_a(s + 1)
                if s == 0:
                    for t in range(8):
                        wpf = work.tile([128, 1024], F32, tag="ob", bufs=4, name=f"wpf{t}")
                        sl = slice(1024 * t, 1024 * (t + 1))
                        nc.sync.dma_start(wpf[:], wp[128 * t:128 * (t + 1), :])
                        nc.scalar.copy(wpb[:, sl], wpf[:])
                phase_c(s)
                phase_d(s)

    _split_multi_waits(nc)
    return nc


_NC = None


def kernel(x, w_attn, w_proj, split_sections):
    global _NC
    if _NC is None:
        _NC = _build()
    x = np.ascontiguousarray(np.asarray(x, dtype=np.float32))
    w_attn = np.ascontiguousarray(np.asarray(w_attn, dtype=np.float32))
    w_proj = np.ascontiguousarray(np.asarray(w_proj, dtype=np.float32))
    in_maps = [
        {"x_sh": x[i * TOK:(i + 1) * TOK], "w_attn": w_attn, "w_proj": w_proj}
        for i in range(NCORES)
    ]
    res = run_bass_kernel_spmd(_NC, in_maps, core_ids=list(range(NCORES)))
    return np.concatenate([res.results[i]["out"] for i in range(NCORES)], axis=0)


if __name__ == "__main__":
    rng = np.random.default_rng(0)
    x = rng.standard_normal((T, C), dtype=np.float32)
    wa = (rng.standard_normal((C, 3 * C), dtype=np.float32) / np.sqrt(C)).astype(np.float32)
    wpj = (rng.standard_normal((C, C), dtype=np.float32) / np.sqrt(C)).astype(np.float32)
    y = kernel(x, wa, wpj, np.arange(1, 32) * 512)
    print("out", y.shape, y.dtype, np.abs(y).mean())
